# revision 11
# baseline (speedup 1.0000x reference)
"""Trainium2 Bass kernel for a LongNet attention block.

Problem: x (1,48,256,256) -> patchify to 16384 tokens of dim 192 ->
4 segments of 4096 tokens -> q/k/v proj + LayerNorm each -> full
attention within each segment -> un-patchify.

Sharding: 2 cores per segment (8 cores, 4 segments). Each core computes
attention for 2048 queries against its segment's full 4096 keys/values.
Softmax is key-order invariant, so the host permutes each core's token
columns so its query half is always columns 0:2048 -> one SPMD program.

Device pipeline per core (matmuls bf16 in / f32 PSUM accumulate):
  1. q/k/v projections from transposed tokens xsT [d, s] with a ones-row
     folding the bias into the matmul. Mean-centering is folded into the
     weights on the host (W' = W - colmean(W)), so projections emerge
     already centered; q+k (or k+v) share one N=384 matmul.
  2. LayerNorm variance via one square-with-accumulate DVE op per
     projection; rsqrt = ScalarE Sqrt + VectorE reciprocal. Sqrt shares
     an activation table set with Copy/Square, Exp has its own ->
     exactly two table loads for the whole kernel.
  3. Normalized q is PE-transposed to [e, s] layout; k is transposed
     UN-scaled and its 1/std is applied later as the exp's per-partition
     scale AP (softmax keys live on partitions there). v stays natural
     with a ones-column appended so the softmax denominator falls out of
     the attention matmul for free.
  4. scoresT = kT.T @ qT per 128-key chunk over query slab-pairs; exp is
     applied by ScalarE straight out of PSUM on [128,1024] tiles with
     scale = r_k/sqrt(D). No row-max subtraction is needed:
     |scores| <= D / sqrt(D) ~= 13.9.
  5. outT[e, sq] accumulates v.T @ p over key chunks in PSUM; the ones
     column of v produces the rowsum row. Host divides and transposes.
"""

import numpy as np
import ml_dtypes

import concourse.bacc as bacc
import concourse.mybir as mybir
import concourse.tile as tile
from concourse.bass_utils import run_bass_kernel_spmd

WS = 2
C = 48
IMG = 256
NS = IMG // WS          # 128
D = C * WS * WS         # 192
S = NS * NS             # 16384
SEG = 4096
G = S // SEG            # 4 segments
NQ = SEG // 2           # 2048 queries per core
NCORES = 8
EPS = 1e-5
SCALE_C = float(D) ** -0.5
SLAB = 512
NKC = SEG // 128        # 32 key chunks
NQC = NQ // 128         # 16 query chunks
VW = 200                # padded per-chunk v width (192 data + ones col @192)

F32 = mybir.dt.float32
BF16 = mybir.dt.bfloat16
FT = mybir.ActivationFunctionType
OP = mybir.AluOpType

_PROGRAM_CACHE = {}


def _build_program(general_gb: bool):
    nc = bacc.Bacc(
        "TRN2",
        target_bir_lowering=False,
        debug=False,
        enable_asserts=False,
    )
    xa = nc.dram_tensor("xa", [128, SEG], BF16, kind="ExternalInput").ap()
    xb = nc.dram_tensor("xb", [128, SEG], BF16, kind="ExternalInput").ap()
    wa = nc.dram_tensor("wa", [128, 3 * D], BF16, kind="ExternalInput").ap()
    wb = nc.dram_tensor("wb", [128, 3 * D], BF16, kind="ExternalInput").ap()
    idn = nc.dram_tensor("idn", [128, 128], BF16, kind="ExternalInput").ap()
    if general_gb:
        gcol = nc.dram_tensor("gcol", [D, 1], F32, kind="ExternalInput").ap()
        bcol = nc.dram_tensor("bcol", [D, 1], F32, kind="ExternalInput").ap()
        gbc = nc.dram_tensor("gbc", [128, D], F32, kind="ExternalInput").ap()
        bbc = nc.dram_tensor("bbc", [128, D], F32, kind="ExternalInput").ap()
    outa = nc.dram_tensor("outa", [128, NQ], F32, kind="ExternalOutput").ap()
    outb = nc.dram_tensor("outb", [65, NQ], F32, kind="ExternalOutput").ap()

    with tile.TileContext(nc) as tc:
        with tc.tile_pool(name="const", bufs=1) as const, \
             tc.tile_pool(name="persist", bufs=1) as persist:
            xa_s = const.tile([128, SEG], BF16)
            nc.sync.dma_start(xa_s, xa)
            xb_s = const.tile([128, SEG], BF16)
            nc.sync.dma_start(xb_s, xb)
            wa_s = const.tile([128, 3 * D], BF16)
            nc.sync.dma_start(wa_s, wa)
            wb_s = const.tile([128, 3 * D], BF16)
            nc.sync.dma_start(wb_s, wb)
            idn_s = const.tile([128, 128], BF16)
            nc.sync.dma_start(idn_s, idn)
            epsc = const.tile([128, 1], F32)
            nc.gpsimd.memset(epsc, EPS)
            if general_gb:
                gca = const.tile([128, 1], F32)
                nc.sync.dma_start(gca, gcol[0:128])
                gcb = const.tile([64, 1], F32)
                nc.sync.dma_start(gcb, gcol[128:192])
                bca = const.tile([128, 1], F32)
                nc.sync.dma_start(bca, bcol[0:128])
                bcb = const.tile([64, 1], F32)
                nc.sync.dma_start(bcb, bcol[128:192])
                gbc_s = const.tile([128, D], F32)
                nc.sync.dma_start(gbc_s, gbc)
                bbc_s = const.tile([128, D], F32)
                nc.sync.dma_start(bbc_s, bbc)

            kT0 = persist.tile([128, SEG], BF16)
            kT1 = persist.tile([128, SEG], BF16)
            qT0 = persist.tile([128, NQ], BF16)
            qT1 = persist.tile([128, NQ], BF16)
            vat = persist.tile([128, NKC, VW], BF16)
            rkc = persist.tile([128, NKC], F32)  # SCALE_C / std_k per key
            nc.gpsimd.memset(kT1[64:128, :], 0.0)
            nc.gpsimd.memset(qT1[64:128, :], 0.0)
            nc.gpsimd.memset(vat[:, :, 192:193], 1.0)

            # ---- Phase 1: projections + LayerNorm (+ q/k transpose) ----
            with tc.tile_pool(name="pp_proj", bufs=2, space="PSUM") as pp_proj, \
                 tc.tile_pool(name="pp_tr", bufs=2, space="PSUM") as pp_tr, \
                 tc.tile_pool(name="ln_sb", bufs=3) as ln_sb, \
                 tc.tile_pool(name="smalls", bufs=4) as smalls:
                for c in range(NKC):
                    csl = slice(c * 128, (c + 1) * 128)
                    has_q = c < NQC
                    # fused projections: [q k] + [v], or [k v]
                    rawA = pp_proj.tile([128, 2 * D], F32, name="rawA")
                    ra_cols = slice(0, 2 * D) if has_q else slice(D, 3 * D)
                    nc.tensor.matmul(rawA, lhsT=xa_s[:, csl], rhs=wa_s[:, ra_cols],
                                     start=True, stop=False)
                    nc.tensor.matmul(rawA, lhsT=xb_s[:, csl], rhs=wb_s[:, ra_cols],
                                     start=False, stop=True)
                    if has_q:
                        rawV = pp_proj.tile([128, D], F32, name="rawV")
                        nc.tensor.matmul(rawV, lhsT=xa_s[:, csl],
                                         rhs=wa_s[:, 2 * D:3 * D],
                                         start=True, stop=False)
                        nc.tensor.matmul(rawV, lhsT=xb_s[:, csl],
                                         rhs=wb_s[:, 2 * D:3 * D],
                                         start=False, stop=True)
                        raws = [("q", rawA[:, 0:D]), ("k", rawA[:, D:2 * D]),
                                ("v", rawV[:, 0:D])]
                    else:
                        raws = [("k", rawA[:, 0:D]), ("v", rawA[:, D:2 * D])]
                    w = len(raws)
                    # centered values to SBUF (bf16) + sum-of-squares
                    ssq = smalls.tile([128, 3], F32, name="ssq")
                    cpres = []
                    for j, (pname, rap) in enumerate(raws):
                        cpre = ln_sb.tile([128, D], BF16, name=f"cpre{pname}")
                        nc.vector.tensor_copy(cpre, rap)
                        cpres.append(cpre)
                        sqd = ln_sb.tile([128, D], BF16, name="sqd")
                        nc.vector.scalar_tensor_tensor(
                            sqd, cpre, 1.0, cpre, OP.mult, OP.mult,
                            accum_out=ssq[:, j:j + 1])
                    # std = sqrt(ssq/D + eps); r = 1/std
                    sds = smalls.tile([128, 3], F32, name="sds")
                    nc.scalar.activation(sds[:, 0:w], ssq[:, 0:w], FT.Sqrt,
                                         scale=1.0 / D, bias=epsc)
                    r3 = smalls.tile([128, 3], F32, name="r3")
                    nc.vector.reciprocal(r3[:, 0:w], sds[:, 0:w])

                    for j, (pname, rap) in enumerate(raws):
                        cpre = cpres[j]
                        rj = r3[:, j:j + 1]
                        if pname == "v":
                            if general_gb:
                                t1 = ln_sb.tile([128, D], F32, name="t1")
                                nc.vector.tensor_scalar(t1, cpre, rj, None, OP.mult)
                                t2 = ln_sb.tile([128, D], F32, name="t2")
                                nc.vector.tensor_tensor(t2, t1, gbc_s, OP.mult)
                                nc.vector.tensor_tensor(
                                    vat[:, c, 0:192], t2, bbc_s, OP.add)
                            else:
                                nc.vector.tensor_scalar(
                                    vat[:, c, 0:192], cpre, rj, None, OP.mult)
                            continue
                        if pname == "k" and not general_gb:
                            # r_k folded into the exp scale later
                            nc.vector.tensor_scalar_mul(
                                rkc[:, c:c + 1], rj, SCALE_C)
                            tsrc = cpre
                        else:
                            tsrc = ln_sb.tile([128, D], BF16, name="tsrc")
                            nc.vector.tensor_scalar(tsrc, cpre, rj, None, OP.mult)
                        tp0 = pp_tr.tile([128, 128], BF16, name="tp0")
                        nc.tensor.transpose(tp0, tsrc[:, 0:128], idn_s)
                        tp1 = pp_tr.tile([64, 128], BF16, name="tp1")
                        nc.tensor.transpose(tp1, tsrc[:, 128:192], idn_s)
                        dT0, dT1 = (qT0, qT1) if pname == "q" else (kT0, kT1)
                        if general_gb:
                            nc.vector.tensor_scalar(
                                dT0[:, csl], tp0, gca, bca, OP.mult, OP.add)
                            nc.vector.tensor_scalar(
                                dT1[0:64, csl], tp1, gcb, bcb, OP.mult, OP.add)
                        else:
                            nc.vector.tensor_copy(dT0[:, csl], tp0)
                            nc.vector.tensor_copy(dT1[0:64, csl], tp1)

            # ---- Phase 2: scores -> exp -> attn@v, per query slab-pair ----
            with tc.tile_pool(name="pp_sc", bufs=2, space="PSUM") as pp_sc, \
                 tc.tile_pool(name="pp_oa", bufs=1, space="PSUM") as pp_oa, \
                 tc.tile_pool(name="pp_ob", bufs=1, space="PSUM") as pp_ob, \
                 tc.tile_pool(name="pt_pool", bufs=3) as pt_pool, \
                 tc.tile_pool(name="ev", bufs=2) as ev:
                for sp in range(NQ // (2 * SLAB)):
                    q0 = slice(2 * sp * SLAB, (2 * sp + 1) * SLAB)
                    q1 = slice((2 * sp + 1) * SLAB, (2 * sp + 2) * SLAB)
                    oA0 = pp_oa.tile([128, SLAB], F32, name="oA0")
                    oA1 = pp_oa.tile([128, SLAB], F32, name="oA1")
                    oB0 = pp_ob.tile([65, SLAB], F32, name="oB0")
                    oB1 = pp_ob.tile([65, SLAB], F32, name="oB1")

                    def emit_sc(c):
                        ksl = slice(c * 128, (c + 1) * 128)
                        sct = pp_sc.tile([128, 2 * SLAB], F32, name="sct")
                        nc.tensor.matmul(sct[:, 0:SLAB], lhsT=kT0[:, ksl],
                                         rhs=qT0[:, q0], start=True, stop=False)
                        nc.tensor.matmul(sct[:, 0:SLAB], lhsT=kT1[:, ksl],
                                         rhs=qT1[:, q0], start=False, stop=True)
                        nc.tensor.matmul(sct[:, SLAB:], lhsT=kT0[:, ksl],
                                         rhs=qT0[:, q1], start=True, stop=False)
                        nc.tensor.matmul(sct[:, SLAB:], lhsT=kT1[:, ksl],
                                         rhs=qT1[:, q1], start=False, stop=True)
                        pt = pt_pool.tile([128, 2 * SLAB], BF16, name="pt")
                        sc_arg = rkc[:, c:c + 1] if not general_gb else SCALE_C
                        nc.scalar.activation(pt, sct, FT.Exp, scale=sc_arg)
                        return pt

                    def emit_av(c, pt):
                        st, sp_ = (c == 0), (c == NKC - 1)
                        nc.tensor.matmul(oA0, lhsT=vat[:, c, 0:128],
                                         rhs=pt[:, 0:SLAB], start=st, stop=sp_)
                        nc.tensor.matmul(oB0, lhsT=vat[:, c, 128:193],
                                         rhs=pt[:, 0:SLAB], start=st, stop=sp_)
                        nc.tensor.matmul(oA1, lhsT=vat[:, c, 0:128],
                                         rhs=pt[:, SLAB:], start=st, stop=sp_)
                        nc.tensor.matmul(oB1, lhsT=vat[:, c, 128:193],
                                         rhs=pt[:, SLAB:], start=st, stop=sp_)

                    # software-pipelined: PE streams scores chunk c+1 while
                    # ScalarE exps chunk c
                    pt_prev = emit_sc(0)
                    for c in range(1, NKC):
                        pt_c = emit_sc(c)
                        emit_av(c - 1, pt_prev)
                        pt_prev = pt_c
                    emit_av(NKC - 1, pt_prev)

                    for qsl, oA, oB in ((q0, oA0, oB0), (q1, oA1, oB1)):
                        ea = ev.tile([128, SLAB], F32, name="ea")
                        nc.vector.tensor_copy(ea, oA)
                        eb = ev.tile([65, SLAB], F32, name="eb")
                        nc.vector.tensor_copy(eb, oB)
                        nc.sync.dma_start(outa[:, qsl], ea)
                        nc.sync.dma_start(outb[:, qsl], eb)

    nc.compile()
    return nc


def _get_program(general_gb: bool):
    key = bool(general_gb)
    if key not in _PROGRAM_CACHE:
        _PROGRAM_CACHE[key] = _build_program(key)
    return _PROGRAM_CACHE[key]


def _patchify(x):
    # (1, C, IMG, IMG) -> (S, D); token s=(i,j), feature d=(c, wi, wj)
    t = x.reshape(C, NS, WS, NS, WS)
    t = np.transpose(t, (1, 3, 0, 2, 4))
    return np.ascontiguousarray(t.reshape(S, D))


def _unpatchify(tokens):
    # (S, D) -> (1, C, IMG, IMG)
    t = tokens.reshape(NS, NS, C, WS, WS)
    t = np.transpose(t, (2, 0, 3, 1, 4))
    return np.ascontiguousarray(t.reshape(1, C, IMG, IMG))


def _prepare(inputs):
    x = np.asarray(inputs["x"], dtype=np.float32)
    Wq = np.asarray(inputs["Wq"], dtype=np.float32)
    Wk = np.asarray(inputs["Wk"], dtype=np.float32)
    Wv = np.asarray(inputs["Wv"], dtype=np.float32)
    bq = np.asarray(inputs["bq"], dtype=np.float32)
    bk = np.asarray(inputs["bk"], dtype=np.float32)
    bv = np.asarray(inputs["bv"], dtype=np.float32)
    gamma = np.asarray(inputs["gamma"], dtype=np.float32)
    beta = np.asarray(inputs["beta"], dtype=np.float32)

    general_gb = not (np.all(gamma == 1.0) and np.all(beta == 0.0))
    nc = _get_program(general_gb)

    bf = ml_dtypes.bfloat16
    xs = _patchify(x)

    # center the projection outputs by folding the per-column mean into
    # the weights: q_centered = x @ (W - colmean W)^T + (b - mean b)
    def centered(W, b):
        Wc = W - W.mean(axis=0, keepdims=True)
        bc = b - b.mean()
        return Wc, bc

    Wqc, bqc = centered(Wq, bq)
    Wkc, bkc = centered(Wk, bk)
    Wvc, bvc = centered(Wv, bv)

    # weight tensors: wa/wb = [WqT | WkT | WvT] split over the contraction
    # dim (192 -> 128 + 64), with the bias as an appended ones-row product
    wa = np.concatenate([Wqc.T[0:128], Wkc.T[0:128], Wvc.T[0:128]], axis=1)
    wb = np.zeros((128, 3 * D), np.float32)
    wb[0:64, 0:D] = Wqc.T[128:192]
    wb[0:64, D:2 * D] = Wkc.T[128:192]
    wb[0:64, 2 * D:3 * D] = Wvc.T[128:192]
    wb[64, 0:D] = bqc
    wb[64, D:2 * D] = bkc
    wb[64, 2 * D:3 * D] = bvc
    wa = wa.astype(bf)
    wb = wb.astype(bf)
    idn = np.eye(128, dtype=bf)

    in_maps = []
    for core in range(NCORES):
        g, h = core // 2, core % 2
        seg = xs[g * SEG:(g + 1) * SEG]
        perm = np.concatenate(
            [seg[h * NQ:(h + 1) * NQ], seg[(1 - h) * NQ:(2 - h) * NQ]], axis=0)
        xsT = perm.T  # (192, 4096)
        xav = np.ascontiguousarray(xsT[0:128]).astype(bf)
        xbv = np.zeros((128, SEG), np.float32)
        xbv[0:64] = xsT[128:192]
        xbv[64] = 1.0
        xbv = xbv.astype(bf)
        im = {"xa": xav, "xb": xbv, "wa": wa, "wb": wb, "idn": idn}
        if general_gb:
            im["gcol"] = gamma.reshape(D, 1).copy()
            im["bcol"] = beta.reshape(D, 1).copy()
            im["gbc"] = np.broadcast_to(gamma, (128, D)).copy()
            im["bbc"] = np.broadcast_to(beta, (128, D)).copy()
        in_maps.append(im)

    return nc, in_maps, general_gb


def _postprocess(res):
    out_tokens = np.empty((S, D), np.float32)
    for core in range(NCORES):
        g, h = core // 2, core % 2
        outa = res.results[core]["outa"]  # (128, NQ) unnormalized outT
        outb = res.results[core]["outb"]  # (65, NQ): rows 0:64 outT, row 64 sums
        o_t = np.concatenate([outa, outb[0:64]], axis=0)  # (192, NQ)
        sums = outb[64]
        out_tokens[g * SEG + h * NQ: g * SEG + (h + 1) * NQ] = (o_t / sums).T

    return _unpatchify(out_tokens)


def kernel(**inputs):
    nc, in_maps, _ = _prepare(inputs)
    res = run_bass_kernel_spmd(nc, in_maps, list(range(NCORES)))
    return _postprocess(res)
